# revision 76
# baseline (speedup 1.0000x reference)
"""Trainium2 Bass kernel for a single transformer block (nn_Block_3212635537783).

Reference computation (B=4, T=2048, C=768, H=12, D=64):
    q/k/v per-head projections of x; scores[t,s] = k[t]@q[s]/sqrt(C) with
    causal mask (s <= t), softmax over s; a[t] = sum_s w[t,s] v[s];
    x = LN1(x + a); x = LN2(x + gelu(x@W1 + b1)@W2 + b2)

Sharding: 8 cores = 4 batches x 2 token-interleaved halves. Core (b, g)
owns rows {g, g+2, ...} of batch b. The stride-2 interleave keeps the
causal workload balanced AND the SPMD program identical across cores
(only input data differs; the +-1 row causal boundary lives in a tiny
per-core mask tile).

On-chip layout is fully "transposed": activations are [C, tokens]
(feature dim on partitions) so attention, layernorm and the MLP never
need an on-chip transpose. Matmul inputs are bf16 (fp32 accumulation);
layernorm statistics use fp32 ones-vector matmuls (fp32r mode).
Softmax denominators ride the A@V matmul as a 65th "ones" value row.
"""

import sys
import types

import numpy as np
import ml_dtypes

B, T, C, H, D = 4, 2048, 768, 12, 64
F = 4 * C            # 3072
P = 128              # partitions
OT = T // 2          # owned tokens per core (1024)
NB_C = C // P        # 6 c-chunks
NB_F = F // P        # 24 hidden chunks
NPAIR = H // 2       # 6 head-pair chunks
EPS = 1e-5
SCALE = float(1.0 / np.sqrt(np.float32(C)))
MASK_NEG = -30000.0
N_CORES = 8
HG = 4               # heads per attention group
N_HG = H // HG       # 3 groups

BF16 = ml_dtypes.bfloat16

_compiled = {}


# --------------------------------------------------------------------------
# environment patches (must live in kernel.py: the grader imports only this
# file). Idempotent.
# --------------------------------------------------------------------------

def _patch_tile_drain():
    """This walrus build rejects >1 sync-wait command on the final Tile
    drain CTRL instruction. Spread the drain's waits across chained
    sync-engine nops (same engine => program order preserved; the
    all-engine barrier after them still gates the semaphore clears)."""
    import concourse.tile as tile_mod
    import concourse.mybir as mybir

    if getattr(tile_mod.TileContext, "_drain_patched", False):
        return

    def patched(self, tick_clock, wait_clock):
        from concourse.vector_clock import ScopedClock

        drain_inst = self.nc.sync.drain()
        wait_clock.add_sem_waits(
            drain_inst.ins, ScopedClock({None: tick_clock.global_clock})
        )
        si = drain_inst.ins.sync_info
        waits = list(si.on_wait) if si else []
        MAXW = 1
        if len(waits) > MAXW:
            si.on_wait = waits[:MAXW]
            rest = waits[MAXW:]
            while rest:
                nop = self.nc.sync.nop(nofuse=True)
                chunk, rest = rest[:MAXW], rest[MAXW:]
                nsi = nop.ins.sync_info
                if nsi is None:
                    nop.ins.sync_info = mybir.SyncInfo(on_wait=chunk, on_update=[])
                else:
                    nsi.on_wait = list(nsi.on_wait) + chunk
        self.nc.all_engine_barrier()
        assert self.sems is not None
        popped = self.nc._tile_sem_poison_stack.pop()
        assert popped is self._sem_poison
        self.nc.clear_and_free_semaphores(list(self.sems.allocated().values()))
        self.nc.all_engine_barrier()

    tile_mod.TileContext._drain_and_barrier = patched
    tile_mod.TileContext._drain_patched = True


def _patch_profile_hook():
    """Optional: register the axon NTFF profiling hook so trace=True works
    (used for timing; harmless no-op if unavailable)."""
    if "antenv.axon_hooks" in sys.modules:
        return
    try:
        sys.path.insert(0, "/root/.axon_site")
        from trn_agent_boot.trn_boot import _ntff_profile_via_ctypes

        hook = _ntff_profile_via_ctypes("/opt/axon/libaxon_pjrt.so")
        mod = types.ModuleType("antenv.axon_hooks")
        mod.get_axon_ntff_profile_hook = lambda: hook
        mod.set_axon_ntff_profile_hook = lambda h: None
        sys.modules["antenv.axon_hooks"] = mod
        import concourse.bass_utils as bu

        bu.upload_artifacts = lambda tmpdir: "local://" + tmpdir
    except Exception:
        pass


# --------------------------------------------------------------------------
# program construction (shared by all 8 cores; SPMD over input data)
# --------------------------------------------------------------------------

def _build_nc():
    import contextlib

    import concourse.bass as bass
    import concourse.mybir as mybir
    from concourse.tile import TileContext

    f32 = mybir.dt.float32
    f32r = mybir.dt.float32r
    bf16 = mybir.dt.bfloat16
    ALU = mybir.AluOpType
    AF = mybir.ActivationFunctionType

    nc = bass.Bass()

    # ---- DRAM I/O ----
    xT = nc.declare_dram_parameter("xT", [C, T], bf16, isOutput=False)
    xTo = nc.declare_dram_parameter("xTo", [C, OT], f32, isOutput=False)
    xTo16 = nc.declare_dram_parameter("xTo16", [C, OT], bf16, isOutput=False)
    wq = nc.declare_dram_parameter("wq", [C, C], bf16, isOutput=False)
    wk = nc.declare_dram_parameter("wk", [C, C], bf16, isOutput=False)
    wv = nc.declare_dram_parameter("wv", [C, C], bf16, isOutput=False)
    w1 = nc.declare_dram_parameter("w1", [C, F], bf16, isOutput=False)
    w2 = nc.declare_dram_parameter("w2", [F, C], bf16, isOutput=False)
    b1r = nc.declare_dram_parameter("b1r", [P, NB_F], f32, isOutput=False)
    b2r = nc.declare_dram_parameter("b2r", [P, NB_C], f32, isOutput=False)
    g1r = nc.declare_dram_parameter("g1r", [P, NB_C], f32, isOutput=False)
    be1r = nc.declare_dram_parameter("be1r", [P, NB_C], f32, isOutput=False)
    g2r = nc.declare_dram_parameter("g2r", [P, NB_C], f32, isOutput=False)
    be2r = nc.declare_dram_parameter("be2r", [P, NB_C], f32, isOutput=False)
    cmask = nc.declare_dram_parameter("cmask", [P, 64], bf16, isOutput=False)
    sel4h = nc.declare_dram_parameter("sel4h", [P, 2 * P], f32, isOutput=False)
    outT = nc.declare_dram_parameter("outT", [C, OT], f32, isOutput=True)

    xT_t = xT[:].rearrange("(n p) t -> n p t", p=P)
    xTo_t = xTo[:].rearrange("(n p) t -> n p t", p=P)
    xTo16_t = xTo16[:].rearrange("(n p) t -> n p t", p=P)
    wq_t = wq[:].rearrange("(n p) c -> n p c", p=P)
    wk_t = wk[:].rearrange("(n p) c -> n p c", p=P)
    wv_t = wv[:].rearrange("(n p) c -> n p c", p=P)
    w1_t = w1[:].rearrange("(n p) f -> n p f", p=P)
    w2_t = w2[:].rearrange("(n p) c -> n p c", p=P)
    outT_t = outT[:].rearrange("(n p) t -> n p t", p=P)

    def r(ap):
        """bitcast fp32 matmul operands to fp32r (full-rate at N>=256)"""
        return ap.bitcast(f32r)

    with TileContext(nc) as tc, contextlib.ExitStack() as ctx:
        const = ctx.enter_context(tc.tile_pool(name="const", bufs=1))
        p_xto = ctx.enter_context(tc.tile_pool(name="xto", bufs=1))
        p_a = ctx.enter_context(tc.tile_pool(name="attn_a", bufs=1))
        p_mlpw = ctx.enter_context(tc.tile_pool(name="mlpw", bufs=1))
        import contextlib as _ctl
        xt_stack = _ctl.ExitStack()
        p_xt = xt_stack.enter_context(tc.tile_pool(name="xt", bufs=1))

        # ---- constants ----
        ones_k = const.tile([P, 1], bf16, tag="ones_k", name="ones_k")
        nc.vector.memset(ones_k, 1.0)
        ones_kf = const.tile([P, 1], f32, tag="ones_kf", name="ones_kf")
        nc.vector.memset(ones_kf, 1.0)
        ones_kr = const.tile([P, 1], f32r, tag="ones_kr", name="ones_kr")
        with nc.allow_low_precision(reason="f32r ones column for LN stats"):
            nc.vector.tensor_copy(ones_kr, ones_kf)
        ones_bf = const.tile([1, P], f32, tag="ones_bf", name="ones_bf")
        nc.vector.memset(ones_bf, 1.0)
        ones_b = const.tile([1, P], f32r, tag="ones_b", name="ones_b")
        with nc.allow_low_precision(reason="f32r ones for 1cyc/row bcast"):
            nc.vector.tensor_copy(ones_b, ones_bf)
        # pair selectors (host-built): sel4[:, g, :] as lhsT routes the
        # denominator-recip row at partition 32*(2g+par) to output rows
        # par*64..par*64+63 (par = 0,1) in one broadcast matmul per pair.
        # Attention-only tiles -> p_xt pool (freed before the MLP phase).
        sel4f = p_xt.tile([P, 2, P], f32, tag="sel4f", name="sel4f")
        nc.scalar.dma_start(out=sel4f, in_=sel4h[:].rearrange("p (g m) -> p g m", g=2))
        sel4 = p_xt.tile([P, 2, P], f32r, tag="sel4", name="sel4")
        with nc.allow_low_precision(reason="f32r selector for den bcast"):
            nc.vector.tensor_copy(sel4, sel4f)
        # denominator staging: head-slot jj's row lives at partition 32*jj
        # (engine partition starts must be 32-aligned); one reciprocal
        # covers all 128 lanes at the cost of one row
        den4 = p_xt.tile([P, 512], f32, tag="den4", name="den4")
        nc.vector.memset(den4, 1.0)
        rec4 = p_xt.tile([P, 512], f32r, tag="rec4", name="rec4")
        eps_t = const.tile([1, 1], f32, tag="eps", name="eps_t")
        nc.vector.memset(eps_t, EPS)
        # dummy exp: pulls the ~2.7us exp table load into the startup DMA
        # wait instead of stalling the first attention softmax
        warm = const.tile([1, 1], f32, tag="warm", name="warm")
        nc.scalar.activation(out=warm, in_=eps_t, func=AF.Exp, scale=1.0)
        msk = const.tile([P, 64], bf16, tag="msk", name="msk")
        nc.scalar.dma_start(out=msk, in_=cmask[:])
        msk2 = bass.AP(
            tensor=msk.tensor, offset=msk.offset,
            ap=[list(msk.ap[0]), [0, 2], list(msk.ap[1])],
        )
        sb_b1 = const.tile([P, NB_F], f32, tag="b1", name="sb_b1")
        nc.scalar.dma_start(out=sb_b1, in_=b1r[:])
        sb_b2 = const.tile([P, NB_C], f32, tag="b2", name="sb_b2")
        nc.scalar.dma_start(out=sb_b2, in_=b2r[:])
        sb_g1 = const.tile([P, NB_C], f32, tag="g1", name="sb_g1")
        nc.scalar.dma_start(out=sb_g1, in_=g1r[:])
        sb_be1 = const.tile([P, NB_C], f32, tag="be1", name="sb_be1")
        nc.scalar.dma_start(out=sb_be1, in_=be1r[:])
        sb_g2 = const.tile([P, NB_C], f32, tag="g2", name="sb_g2")
        nc.scalar.dma_start(out=sb_g2, in_=g2r[:])
        sb_be2 = const.tile([P, NB_C], f32, tag="be2", name="sb_be2")
        nc.scalar.dma_start(out=sb_be2, in_=be2r[:])

        # ---- persistent activations ----
        # combined activation tiles: one DMA dispatch each (per-chunk
        # dispatches cost ~0.7us of queue time apiece at startup)
        xt_all = p_xt.tile([P, NB_C, T], bf16, tag="xt", name="xt")
        xto_all = p_xto.tile([P, NB_C, OT], f32, tag="xto", name="xto")
        xto16_all = p_xt.tile([P, NB_C, OT], bf16, tag="xto16", name="xto16")
        sb_xt = [xt_all[:, k, :] for k in range(NB_C)]
        sb_xto = [xto_all[:, k, :] for k in range(NB_C)]
        sb_xto16 = [xto16_all[:, k, :] for k in range(NB_C)]

        # attention output a^T, bf16 [128, OT] per pair-chunk
        sb_a = [
            p_a.tile([P, OT], bf16, tag=f"a{pc}", name=f"a{pc}")
            for pc in range(NPAIR)
        ]

        # MLP weight tiles: allocate now (address space), DMA later --
        # issuing these loads early would queue 9.4MB ahead of the weights
        # the first projections need and stall the PE for ~60us.
        sb_w1 = [
            p_mlpw.tile([P, F], bf16, tag=f"w1_{k}", name=f"w1_{k}")
            for k in range(NB_C)
        ]
        # w2 lives in a phase-B pool (opened after the attention tiles are
        # freed): its 36KB/partition would otherwise cap attention buffers

        # rotating projection-weight pool: one combined DMA per (w, pc),
        # prefetched one group ahead so group starts never wait on HBM
        # (lives on xt_stack: freed with the other attention-only tiles)
        p_wg = xt_stack.enter_context(tc.tile_pool(name="wqkv", bufs=2))
        wq_r = wq[:].rearrange("(k p) c -> p k c", p=P)
        wk_r = wk[:].rearrange("(k p) c -> p k c", p=P)
        wv_r = wv[:].rearrange("(k p) c -> p k c", p=P)

        def load_gw(hg):
            pcs_ = [hg * (HG // 2) + i for i in range(HG // 2)]
            wt = {}
            for i, pc in enumerate(pcs_):
                t = p_wg.tile([P, NB_C, P], bf16, tag=f"wk{i}",
                              name=f"wk{hg}_{i}")
                nc.sync.dma_start(out=t, in_=wk_r[:, :, pc * P : (pc + 1) * P])
                wt["k", pc] = t
            for i, pc in enumerate(pcs_):
                t = p_wg.tile([P, NB_C, P], bf16, tag=f"wq{i}",
                              name=f"wq{hg}_{i}")
                nc.sync.dma_start(out=t, in_=wq_r[:, :, pc * P : (pc + 1) * P])
                wt["q", pc] = t
            t = p_wg.tile([P, NB_C, HG * D], bf16, tag="wv", name=f"wv{hg}")
            d0 = 2 * pcs_[0] * D
            nc.sync.dma_start(out=t, in_=wv_r[:, :, d0 : d0 + HG * D])
            wt["v"] = t
            return wt

        # startup: group-0 weights race down the sync queue while the bulk
        # activation loads stream on the (otherwise idle) gpsimd DMA queue;
        # the k projection only needs the small xTo16 load, so it runs first
        gw = {0: load_gw(0)}
        nc.gpsimd.dma_start(
            out=xto16_all, in_=xTo16[:].rearrange("(k p) t -> p k t", p=P)
        )
        nc.gpsimd.dma_start(
            out=xt_all, in_=xT[:].rearrange("(k p) t -> p k t", p=P)
        )
        nc.gpsimd.dma_start(
            out=xto_all, in_=xTo[:].rearrange("(k p) t -> p k t", p=P)
        )

        # deferred normalize-finish closures (see norm_a/norm_b below)
        norm_q = []

        def flush_norm(psum_pool):
            while norm_q:
                norm_q.pop(0)(psum_pool)

        # ============================================================
        # Phase A: attention, in head groups of HG
        # ============================================================
        for hg in range(N_HG):
            pcs = [hg * (HG // 2) + i for i in range(HG // 2)]
            heads = [2 * pc + j for pc in pcs for j in range(2)]

            with contextlib.ExitStack() as gctx:
                p_qk = gctx.enter_context(tc.tile_pool(name=f"qk{hg}", bufs=1))
                p_v = gctx.enter_context(tc.tile_pool(name=f"v{hg}", bufs=1))
                p_ps = gctx.enter_context(
                    tc.tile_pool(name=f"ps{hg}", bufs=1, space="PSUM")
                )
                p_e = gctx.enter_context(tc.tile_pool(name=f"e{hg}", bufs=1))

                wt = gw.pop(hg)
                _sc_p = nc.enter_named_scope(f"proj{hg}", False)
                # ---- projections for this group ----
                q_t, k_t = {}, {}
                for pc in pcs:
                    q_t[pc] = p_qk.tile([P, T], bf16, tag=f"q{pc}", name=f"q{pc}")
                    k_t[pc] = p_qk.tile([P, OT], bf16, tag=f"k{pc}", name=f"k{pc}")
                # v for 4 heads: [128, 4, 65] per s-chunk (col 64 of each
                # head-slot = 1.0 for the softmax-denominator value row)
                v4 = []
                for sc in range(T // P):
                    vt = p_v.tile(
                        [P, HG, 65], bf16, tag=f"v4_{sc}", name=f"v4_{sc}"
                    )
                    nc.vector.memset(vt[:, :, 64:65], 1.0)
                    v4.append(vt)

                # k projection first: it only needs the small xTo16 load
                for pc in pcs:
                    wkl = wt["k", pc]
                    for t2 in range(OT // 512):
                        ps = p_ps.tile(
                            [P, 512], f32, tag="ps", bufs=2, name="ps_prk"
                        )
                        for k in range(NB_C):
                            nc.tensor.matmul(
                                ps,
                                wkl[:, k, :],
                                sb_xto16[k][:, t2 * 512 : (t2 + 1) * 512],
                                start=(k == 0),
                                stop=(k == NB_C - 1),
                            )
                        nc.vector.tensor_copy(
                            k_t[pc][:, t2 * 512 : (t2 + 1) * 512], ps
                        )
                for pc in pcs:
                    # q projection: full T
                    wql = wt["q", pc]
                    for t4 in range(T // 512):
                        ps = p_ps.tile(
                            [P, 512], f32, tag="ps", bufs=2, name="ps_prj"
                        )
                        for k in range(NB_C):
                            nc.tensor.matmul(
                                ps,
                                wql[:, k, :],
                                sb_xt[k][:, t4 * 512 : (t4 + 1) * 512],
                                start=(k == 0),
                                stop=(k == NB_C - 1),
                            )
                        nc.vector.tensor_copy(
                            q_t[pc][:, t4 * 512 : (t4 + 1) * 512], ps
                        )

                # v projection: full T, into per-head [128, 65] tiles
                wvl = wt["v"]
                for sc in range(T // P):
                    ps = p_ps.tile(
                        [P, HG * D], f32, tag="ps", bufs=2, name="ps_v"
                    )
                    for k in range(NB_C):
                        nc.tensor.matmul(
                            ps,
                            sb_xt[k][:, sc * P : (sc + 1) * P],
                            wvl[:, k, :],
                            start=(k == 0),
                            stop=(k == NB_C - 1),
                        )
                    nc.vector.tensor_copy(
                        v4[sc][:, :, 0:64],
                        ps[:].rearrange("p (h d) -> p h d", h=HG),
                    )

                nc.leave_named_scope(f"proj{hg}", _sc_p[0], False)
                if hg == 1:
                    for k in range(NB_C):
                        nc.sync.dma_start(out=sb_w1[k], in_=w1_t[k])
                if hg + 1 < N_HG:
                    gw[hg + 1] = load_gw(hg + 1)
                _sc_a = nc.enter_named_scope(f"attn{hg}", False)
                # ---- attention ----
                for tb in range(2):
                    nsc = 8 * tb + 8          # s-chunks for this own-block
                    av = {}
                    for h in heads:
                        av[h] = p_ps.tile(
                            [P, 512], f32, tag=f"av{h % HG}", name=f"av{h}"
                        )

                    def issue_av(pc, sc, c0, et, _nsc=nsc, _av=av):
                        for par in range(2):
                            h = 2 * pc + par
                            jj = heads.index(h)
                            nc.tensor.matmul(
                                _av[h][0:65, c0:512],
                                v4[sc][:, jj, :],
                                et[:, par, c0:512],
                                start=(sc == 0),
                                stop=(sc == _nsc - 1),
                            )

                    # software pipeline: AV matmuls for s-chunk sc issue
                    # after the NEXT chunk's score matmuls, so the in-order
                    # PE queue never stalls on the exp (ACT) latency.
                    pend = []
                    for sc in range(nsc):
                        c0 = max(0, 64 * sc - 512 * tb)   # first live t-col
                        for pc in pcs:
                            ps = p_ps.tile(
                                [P, 2, 512], f32, tag="ps", bufs=2,
                                name="ps_sc",
                            )
                            for par in range(2):
                                # K=64 row-tiled pair: both heads' score
                                # matmuls run concurrently on the PE array
                                nc.tensor.matmul(
                                    ps[:, par, c0:512],
                                    q_t[pc][par * 64 : par * 64 + 64,
                                            sc * P : (sc + 1) * P],
                                    k_t[pc][par * 64 : par * 64 + 64,
                                            tb * 512 + c0 : (tb + 1) * 512],
                                    start=True,
                                    stop=True,
                                    tile_position=(par * 64, 0),
                                )
                            et = p_e.tile(
                                [P, 2, 512], bf16, tag="exp", bufs=4, name="et"
                            )
                            nc.scalar.activation(
                                out=et[:, :, c0:512],
                                in_=ps[:, :, c0:512],
                                func=AF.Exp,
                                scale=SCALE,
                            )
                            if sc >= 8 * tb:   # causal boundary stripe
                                # multiplicative {0,1} mask post-exp: keeps
                                # the DVE op bf16 + SBUF-only (fast path)
                                nc.vector.tensor_tensor(
                                    et[:, :, c0 : c0 + 64],
                                    et[:, :, c0 : c0 + 64],
                                    msk2[:, :, 0:64],
                                    ALU.mult,
                                )
                            pend.append((pc, sc, c0, et))
                        while pend and pend[0][1] < sc:
                            issue_av(*pend.pop(0))
                        if sc == 6:
                            # finish the PREVIOUS block's normalize here:
                            # its reciprocal is long done, so the broadcast
                            # matmuls slot into the PE stream with no stall
                            flush_norm(p_ps)
                    for item in pend:
                        issue_av(*item)
                    # normalize part A (at block end): evacuate raw AV to
                    # sb_a (bf16) and the denominator rows into den4, then
                    # one batched reciprocal. This frees the AV psum banks
                    # within ~2us so the next block's accumulation starts
                    # immediately; the finish (norm_b) is deferred.
                    for jj, h in enumerate(heads):
                        pc, par = h // 2, h % 2
                        a_dst = sb_a[pc][par * 64 : par * 64 + 64,
                                         tb * 512 : (tb + 1) * 512]
                        d_dst = den4[32 * jj : 32 * jj + 1, :]
                        # split the 8 evacuation copies across ACT and DVE
                        # so each AV psum bank frees within ~0.7us of its
                        # last matmul (the next block's accumulation reuses
                        # the same 4 banks)
                        if jj < 2:
                            nc.scalar.copy(a_dst, av[h][0:64, 0:512])
                            nc.vector.tensor_copy(d_dst, av[h][64:65, 0:512])
                        else:
                            nc.vector.tensor_copy(a_dst, av[h][0:64, 0:512])
                            nc.scalar.copy(d_dst, av[h][64:65, 0:512])
                    with nc.allow_low_precision(reason="f32r recip bcast"):
                        nc.vector.reciprocal(rec4, den4)

                    def norm_b(psum_pool, _pcs=pcs, _tb=tb):
                        for g, pc in enumerate(_pcs):
                            den_ps = psum_pool.tile(
                                [P, 512], f32, tag="ps", bufs=2, name="den_ps"
                            )
                            nc.tensor.matmul(
                                den_ps, sel4[:, g, :], rec4,
                                start=True, stop=True,
                            )
                            den_sb = p_xt.tile(
                                [P, 512], bf16, tag="den_sb", bufs=4,
                                name="den_sb",
                            )
                            nc.scalar.copy(den_sb, den_ps)
                            sl_c = slice(_tb * 512, (_tb + 1) * 512)
                            for par in range(2):
                                sl_r = slice(par * 64, par * 64 + 64)
                                nc.vector.tensor_tensor(
                                    sb_a[pc][sl_r, sl_c],
                                    sb_a[pc][sl_r, sl_c],
                                    den_sb[sl_r, :],
                                    ALU.mult,
                                )
                            # fold the LN1 residual in here too: sb_a
                            # becomes x + a while attention still runs,
                            # so phase B starts straight at the LN1 stats
                            nc.vector.tensor_tensor(
                                sb_a[pc][:, sl_c],
                                sb_a[pc][:, sl_c],
                                sb_xto[pc][:, sl_c],
                                ALU.add,
                            )

                    norm_q.append(norm_b)

                # group ends: finish this group's tb=1 normalize inside the
                # group (needs an open PSUM pool); only the last group pays
                # a short recip wait here
                if hg + 1 == N_HG:
                    flush_norm(p_ps)
                nc.leave_named_scope(f"attn{hg}", _sc_a[0], False)

        xt_stack.close()   # free xT / xTo16 before the MLP pools open

        # ============================================================
        # Phase B: residual + LN1 + MLP + residual + LN2, per tb
        # ============================================================
        with contextlib.ExitStack() as mctx:
            mctx.enter_context(nc.named_scope("mlp"))
            p_w2 = mctx.enter_context(tc.tile_pool(name="w2p", bufs=1))
            sb_w2 = [
                p_w2.tile([P, C], bf16, tag=f"w2_{m}", name=f"w2_{m}")
                for m in range(NB_F)
            ]
            for m in range(NB_F):
                nc.sync.dma_start(out=sb_w2[m], in_=w2_t[m])
            p_r1 = mctx.enter_context(tc.tile_pool(name="r1", bufs=1))
            p_ln = mctx.enter_context(tc.tile_pool(name="ln", bufs=1))
            p_tmp = mctx.enter_context(tc.tile_pool(name="tmp", bufs=1))
            p_st = mctx.enter_context(tc.tile_pool(name="st", bufs=1))
            p_psm = mctx.enter_context(
                tc.tile_pool(name="psm", bufs=1, space="PSUM")
            )
            p_h = mctx.enter_context(tc.tile_pool(name="hsb", bufs=1))

            # r1 lives as f32r so the LN mean-stats matmul can consume it
            # at full PE rate without an extra rounding copy (f32r keeps
            # 19 mantissa bits - ~1e-4 relative, far inside tolerance)
            r1 = [
                p_r1.tile([P, OT], f32r, tag=f"r1_{c}", name=f"r1_{c}")
                for c in range(NB_C)
            ]
            ln1 = [
                p_ln.tile([P, OT], bf16, tag=f"ln1_{c}", name=f"ln1_{c}")
                for c in range(NB_C)
            ]

            def layer_norm_stats(src_tiles, ones_st=ones_kr, W=512, off=0):
                """transposed LN part 1: stats via ones-matmuls, then the
                DVE/ACT scalar chain ending in the (slow) reciprocal.
                W/off select a column panel (the reciprocal cost scales
                with W, so narrow panels pipeline). Returns (mu, rsg)."""
                mu_ps = p_psm.tile([1, 512], f32, tag="lnst", bufs=2, name="mu_ps")
                sq_ps = p_psm.tile([1, 512], f32, tag="lnst", bufs=2, name="sq_ps")
                for c in range(NB_C):
                    # squares on ACT (Square lives in every table set);
                    # mean stats straight off the source matmul
                    s = p_tmp.tile([P, 512], bf16, tag="sqt", bufs=2, name="sqt")
                    nc.scalar.activation(
                        out=s[:, 0:W], in_=src_tiles[c][:, off : off + W],
                        func=AF.Square, scale=1.0,
                    )
                    with nc.allow_low_precision(reason="f32r LN mean stats"):
                        nc.tensor.matmul(
                            mu_ps[:, 0:W], ones_st,
                            src_tiles[c][:, off : off + W],
                            start=(c == 0), stop=(c == NB_C - 1),
                        )
                    nc.tensor.matmul(
                        sq_ps[:, 0:W], ones_k, s[:, 0:W],
                        start=(c == 0), stop=(c == NB_C - 1),
                    )
                mu = p_st.tile([1, 512], f32r, tag="mu_s", bufs=2, name="mu")
                with nc.allow_low_precision(reason="f32r stats for 1cyc/row bcast"):
                    nc.vector.tensor_scalar_mul(
                        mu[:, 0:W], mu_ps[:, 0:W], 1.0 / C
                    )
                sq = p_st.tile([1, 512], f32, tag="sq_s", bufs=2, name="sq")
                nc.vector.tensor_scalar_mul(sq[:, 0:W], sq_ps[:, 0:W], 1.0 / C)
                var = p_st.tile([1, 512], f32, tag="var", bufs=2, name="var")
                nc.vector.tensor_tensor(
                    var[:, 0:W], mu[:, 0:W], mu[:, 0:W], ALU.mult
                )
                nc.vector.tensor_tensor(
                    var[:, 0:W], sq[:, 0:W], var[:, 0:W], ALU.subtract
                )
                sd = p_st.tile([1, 512], f32, tag="sd", bufs=2, name="sd")
                nc.scalar.activation(
                    out=sd[:, 0:W], in_=var[:, 0:W], func=AF.Sqrt,
                    bias=eps_t, scale=1.0,
                )
                rsg = p_st.tile([1, 512], f32r, tag="rsg", bufs=2, name="rsg")
                with nc.allow_low_precision(reason="f32r rsig for 1cyc/row bcast"):
                    nc.vector.reciprocal(rsg[:, 0:W], sd[:, 0:W])
                return mu, rsg

            def layer_norm_finish(src_tiles, mu, rsg, gt, bt, out_cb,
                                  split=False, W=512, off=0):
                """transposed LN part 2: broadcast matmuls (issue these only
                once the reciprocal has had time to run, or the PE queue
                stalls behind them) + per-chunk normalize. split=True farms
                part of the d1 chain to GPSIMD for the tail."""
                mu_b = p_psm.tile([P, 512], f32, tag="lnbc", bufs=2, name="mu_b")
                nc.tensor.matmul(
                    mu_b[:, 0:W], ones_b, mu[:, 0:W], start=True, stop=True
                )
                rs_b = p_psm.tile([P, 512], f32, tag="lnbc", bufs=2, name="rs_b")
                nc.tensor.matmul(
                    rs_b[:, 0:W], ones_b, rsg[:, 0:W], start=True, stop=True
                )
                mu_bs = p_tmp.tile([P, 512], f32, tag="mu_bs", bufs=2, name="mu_bs")
                nc.scalar.copy(mu_bs[:, 0:W], mu_b[:, 0:W])
                rs_bs = p_tmp.tile([P, 512], f32, tag="rs_bs", bufs=2, name="rs_bs")
                nc.scalar.copy(rs_bs[:, 0:W], rs_b[:, 0:W])
                for c in range(NB_C):
                    eng = nc.gpsimd if (split and c % 3 == 2) else nc.vector
                    d1 = p_tmp.tile([P, 512], f32, tag="d1", bufs=3, name="d1")
                    eng.tensor_tensor(
                        d1[:, 0:W], src_tiles[c][:, off : off + W],
                        mu_bs[:, 0:W], ALU.subtract,
                    )
                    eng.tensor_tensor(
                        d1[:, 0:W], d1[:, 0:W], rs_bs[:, 0:W], ALU.mult
                    )
                    out_cb(c, d1[:, 0:W], gt, bt)

            def layer_norm_T(src_tiles, gt, bt, out_cb, split=False):
                mu, rsg = layer_norm_stats(src_tiles)
                layer_norm_finish(src_tiles, mu, rsg, gt, bt, out_cb,
                                  split=split)

            # LN1 for BOTH halves first (the x + a residual was already
            # folded into sb_a during attention): stats for both halves are
            # issued before either finish so the broadcast matmuls never
            # sit in the PE queue waiting on the 3.3us reciprocal.
            ln1_parts = []
            for tb in range(2):
                sl = slice(tb * 512, (tb + 1) * 512)
                r1v = [sb_a[c][:, sl] for c in range(NB_C)]

                def ln1_out(c, d2, gt, bt, _sl=sl):
                    nc.vector.tensor_scalar(
                        out=ln1[c][:, _sl], in0=d2,
                        scalar1=gt[:, c : c + 1], scalar2=bt[:, c : c + 1],
                        op0=ALU.mult, op1=ALU.add,
                    )
                    # f32r copy for the post-LN residual (r1 is dead: reuse)
                    with nc.allow_low_precision(reason="f32r residual"):
                        nc.gpsimd.tensor_scalar(
                            out=r1[c][:, _sl], in0=d2,
                            scalar1=gt[:, c : c + 1], scalar2=bt[:, c : c + 1],
                            op0=ALU.mult, op1=ALU.add,
                        )

                ln1_parts.append(
                    (r1v, layer_norm_stats(r1v, ones_st=ones_k), ln1_out)
                )
            for r1v_, (mu_, rsg_), cb_ in ln1_parts:
                layer_norm_finish(r1v_, mu_, rsg_, sb_g1, sb_be1, cb_)

            # LN2(tb) is deferred until after pass1(tb+1)'s matmuls are in
            # the PE queue: its stats matmuls wait on the GPSIMD y-adds, and
            # issuing them eagerly would stall everything behind them. The
            # finish (broadcast matmuls) waits further still, so the PE
            # never queues behind the reciprocal.
            ln2_pend = None
            ln2_stats = None
            for tb in range(2):
                sl = slice(tb * 512, (tb + 1) * 512)

                # MLP pass 1: h[m] = gelu(W1.T ln1 + b1) -> SBUF
                h_sb = []
                for m in range(NB_F):
                    h_ps = p_psm.tile([P, 512], f32, tag="h_ps", bufs=2, name="h_ps")
                    for k in range(NB_C):
                        nc.tensor.matmul(
                            h_ps,
                            sb_w1[k][:, m * P : (m + 1) * P],
                            ln1[k][:, sl],
                            start=(k == 0),
                            stop=(k == NB_C - 1),
                        )
                    hs = p_h.tile([P, 512], bf16, tag=f"h{m}", name=f"h{m}")
                    nc.scalar.activation(
                        out=hs, in_=h_ps, func=AF.Gelu,
                        bias=sb_b1[:, m : m + 1], scale=1.0,
                    )
                    h_sb.append(hs)

                # LN2(prev) stats AFTER the last gelu: its Sqrt would
                # otherwise force two ~2.7us ACT table-set reloads in the
                # middle of this pass's gelu stream (Square needs no
                # switch, so the stats matmuls were free to wait)
                if ln2_pend is not None:
                    ln2_stats = layer_norm_stats(ln2_pend[0])

                # MLP pass 2: y[c] = sum_m W2[m,c].T h[m];  r2 = y + b2 + r1
                r2v = []
                for c in range(NB_C):
                    y_ps = p_psm.tile([P, 512], f32, tag="y_ps", bufs=2, name="y_ps")
                    for m in range(NB_F):
                        nc.tensor.matmul(
                            y_ps,
                            sb_w2[m][:, c * P : (c + 1) * P],
                            h_sb[m],
                            start=(m == 0),
                            stop=(m == NB_F - 1),
                        )
                    y_sb = p_h.tile([P, 512], f32, tag="y_sb", bufs=1, name="y_sb")
                    nc.scalar.activation(
                        out=y_sb, in_=y_ps, func=AF.Identity,
                        bias=sb_b2[:, c : c + 1], scale=1.0,
                    )
                    # alternate engines: this add chain gates the LN2 stats
                    eng = nc.vector if c % 2 == 0 else nc.gpsimd
                    with nc.allow_low_precision(reason="f32r residual"):
                        eng.tensor_tensor(
                            r1[c][:, sl], y_sb, r1[c][:, sl], ALU.add
                        )
                    r2v.append(r1[c][:, sl])
                    if c == 3 and ln2_pend is not None:
                        # finish once the reciprocal has had ~3 matmul
                        # groups of time to complete
                        layer_norm_finish(
                            ln2_pend[0], *ln2_stats, sb_g2, sb_be2,
                            ln2_pend[1],
                        )
                        ln2_pend = None

                def ln2_out(c, d2, gt, bt, _sl=sl, _last=(tb == 1)):
                    # stage the output in the long-dead xto residual tile
                    # (same shape/dtype; each (c, half) region is distinct,
                    # so there is no buffer rotation to wait on)
                    o = sb_xto[c][:, _sl]
                    eng = nc.gpsimd if (_last and c % 2 == 1) else nc.vector
                    eng.tensor_scalar(
                        out=o, in0=d2,
                        scalar1=gt[:, c : c + 1], scalar2=bt[:, c : c + 1],
                        op0=ALU.mult, op1=ALU.add,
                    )
                    dma = nc.gpsimd if (_last and c % 2 == 1) else nc.sync
                    dma.dma_start(out=outT_t[c][:, _sl], in_=o)

                ln2_pend = (r2v, ln2_out)
            # final half: no more matmuls to hide behind. Process it as two
            # 256-column panels: each panel's reciprocal is half as long
            # and hides under the other panel's stats, and the d1/output
            # chains split across DVE and GPSIMD.
            last_r2v, last_cb = ln2_pend
            st_p = [
                layer_norm_stats(last_r2v, W=256, off=o) for o in (0, 256)
            ]
            for i, o in enumerate((0, 256)):
                def cb_panel(c, d2, gt, bt, _o=o):
                    last_cb(c, d2, gt, bt,
                            _sl=slice(512 + _o, 512 + _o + 256))

                layer_norm_finish(last_r2v, *st_p[i], sb_g2, sb_be2,
                                  cb_panel, split=True, W=256, off=o)

    return nc


def _spill_excess_waits(nc, maxw=2):
    """walrus (this build) caps sync-wait commands per instruction. Move
    excess waits onto freshly inserted same-engine nops placed immediately
    before the over-limit instruction (same engine stream => the waits
    still complete before it executes)."""
    import copy

    import concourse.bass as bass
    import concourse.mybir as mybir

    scratch = bass.Bass()
    tpl = scratch.sync.nop(nofuse=True).ins
    ctr = [0]

    def mknop(engine, waits):
        n = copy.deepcopy(tpl)
        ctr[0] += 1
        n.name = f"I-spill{ctr[0]}"
        n.engine = engine
        n.sync_info = mybir.SyncInfo(on_wait=list(waits), on_update=[])
        return n

    fn = nc.m.functions[0]
    for bb in fn.blocks:
        changed = False
        out = []
        for inst in bb.instructions:
            si = inst.sync_info
            waits = list(si.on_wait) if si and si.on_wait else []
            nupd = len(si.on_update) if si and si.on_update else 0
            lim = max(0, maxw - nupd)   # waits + updates <= maxw total
            if len(waits) > lim:
                keep = waits[-lim:] if lim else []
                rest = waits[: len(waits) - lim]
                while rest:
                    chunk, rest = rest[:1], rest[1:]
                    out.append(mknop(inst.engine, chunk))
                si.on_wait = keep
                changed = True
            out.append(inst)
        if changed:
            bb.instructions = out


def _get_nc():
    if "nc" not in _compiled:
        _patch_tile_drain()
        _patch_profile_hook()
        nc = _build_nc()
        _spill_excess_waits(nc, maxw=2)
        _compiled["nc"] = nc
    return _compiled["nc"]


# --------------------------------------------------------------------------
# host-side sharding
# --------------------------------------------------------------------------

def _make_in_maps(x, Wq, Wk, Wv, ln1_g, ln1_b, W1, b1, W2, b2, ln2_g, ln2_b):
    x = np.asarray(x, np.float32)
    wq_s = np.ascontiguousarray(
        np.asarray(Wq, np.float32).transpose(1, 0, 2).reshape(C, C)
    ).astype(BF16)
    wk_s = np.ascontiguousarray(
        np.asarray(Wk, np.float32).transpose(1, 0, 2).reshape(C, C)
    ).astype(BF16)
    wv_s = np.ascontiguousarray(
        np.asarray(Wv, np.float32).transpose(1, 0, 2).reshape(C, C)
    ).astype(BF16)
    w1b = np.asarray(W1, np.float32).astype(BF16)
    w2b = np.asarray(W2, np.float32).astype(BF16)
    b1r = np.ascontiguousarray(np.asarray(b1, np.float32).reshape(NB_F, P).T)
    b2r = np.ascontiguousarray(np.asarray(b2, np.float32).reshape(NB_C, P).T)
    g1r = np.ascontiguousarray(np.asarray(ln1_g, np.float32).reshape(NB_C, P).T)
    be1r = np.ascontiguousarray(np.asarray(ln1_b, np.float32).reshape(NB_C, P).T)
    g2r = np.ascontiguousarray(np.asarray(ln2_g, np.float32).reshape(NB_C, P).T)
    be2r = np.ascontiguousarray(np.asarray(ln2_b, np.float32).reshape(NB_C, P).T)

    in_maps = []
    for core in range(N_CORES):
        b, g = core // 2, core % 2
        xb = x[b]                                # [T, C]
        xTa = np.ascontiguousarray(xb.T)         # [C, T]
        own = np.arange(g, T, 2)
        xo = np.ascontiguousarray(xb[own].T)     # [C, OT] f32
        ii = np.arange(P)[:, None]
        mm = np.arange(64)[None, :]
        cm = np.where(ii <= 2 * mm + g, 1.0, 0.0).astype(BF16)
        # sel[p, g, m] = 1 iff p == 32*(2g+par) and m in par's 64-row block
        sel = np.zeros((P, 2, P), np.float32)
        for g in range(2):
            for par in range(2):
                sel[32 * (2 * g + par), g, 64 * par : 64 * par + 64] = 1.0
        sel = sel.reshape(P, 2 * P)
        in_maps.append(
            {
                "xT": xTa.astype(BF16),
                "xTo": xo,
                "xTo16": xo.astype(BF16),
                "wq": wq_s,
                "wk": wk_s,
                "wv": wv_s,
                "w1": w1b,
                "w2": w2b,
                "b1r": b1r,
                "b2r": b2r,
                "g1r": g1r,
                "be1r": be1r,
                "g2r": g2r,
                "be2r": be2r,
                "cmask": cm,
                "sel4h": sel,
            }
        )
    return in_maps


def _assemble(results):
    out = np.empty((B, T, C), np.float32)
    for core in range(N_CORES):
        b, g = core // 2, core % 2
        own = np.arange(g, T, 2)
        out[b, own, :] = results[core]["outT"].T
    return out


def kernel(_trace=False, **inputs):
    from concourse.bass_utils import run_bass_kernel_spmd

    nc = _get_nc()
    in_maps = _make_in_maps(**inputs)
    res = run_bass_kernel_spmd(nc, in_maps, list(range(N_CORES)), trace=_trace)
    out = _assemble(res.results)
    if _trace:
        return out, res
    return out



# revision 80
# speedup vs baseline: 1.0154x; 1.0154x over previous
"""Trainium2 Bass kernel for a single transformer block (nn_Block_3212635537783).

Reference computation (B=4, T=2048, C=768, H=12, D=64):
    q/k/v per-head projections of x; scores[t,s] = k[t]@q[s]/sqrt(C) with
    causal mask (s <= t), softmax over s; a[t] = sum_s w[t,s] v[s];
    x = LN1(x + a); x = LN2(x + gelu(x@W1 + b1)@W2 + b2)

Sharding: 8 cores = 4 batches x 2 token-interleaved halves. Core (b, g)
owns rows {g, g+2, ...} of batch b. The stride-2 interleave keeps the
causal workload balanced AND the SPMD program identical across cores
(only input data differs; the +-1 row causal boundary lives in a tiny
per-core mask tile).

On-chip layout is fully "transposed": activations are [C, tokens]
(feature dim on partitions) so attention, layernorm and the MLP never
need an on-chip transpose. Matmul inputs are bf16 (fp32 accumulation);
layernorm statistics use fp32 ones-vector matmuls (fp32r mode).
Softmax denominators ride the A@V matmul as a 65th "ones" value row.
"""

import sys
import types

import numpy as np
import ml_dtypes

B, T, C, H, D = 4, 2048, 768, 12, 64
F = 4 * C            # 3072
P = 128              # partitions
OT = T // 2          # owned tokens per core (1024)
NB_C = C // P        # 6 c-chunks
NB_F = F // P        # 24 hidden chunks
NPAIR = H // 2       # 6 head-pair chunks
EPS = 1e-5
SCALE = float(1.0 / np.sqrt(np.float32(C)))
MASK_NEG = -30000.0
N_CORES = 8
HG = 4               # heads per attention group
N_HG = H // HG       # 3 groups

BF16 = ml_dtypes.bfloat16

_compiled = {}


# --------------------------------------------------------------------------
# environment patches (must live in kernel.py: the grader imports only this
# file). Idempotent.
# --------------------------------------------------------------------------

def _patch_tile_drain():
    """This walrus build rejects >1 sync-wait command on the final Tile
    drain CTRL instruction. Spread the drain's waits across chained
    sync-engine nops (same engine => program order preserved; the
    all-engine barrier after them still gates the semaphore clears)."""
    import concourse.tile as tile_mod
    import concourse.mybir as mybir

    if getattr(tile_mod.TileContext, "_drain_patched", False):
        return

    def patched(self, tick_clock, wait_clock):
        from concourse.vector_clock import ScopedClock

        drain_inst = self.nc.sync.drain()
        wait_clock.add_sem_waits(
            drain_inst.ins, ScopedClock({None: tick_clock.global_clock})
        )
        si = drain_inst.ins.sync_info
        waits = list(si.on_wait) if si else []
        MAXW = 1
        if len(waits) > MAXW:
            si.on_wait = waits[:MAXW]
            rest = waits[MAXW:]
            while rest:
                nop = self.nc.sync.nop(nofuse=True)
                chunk, rest = rest[:MAXW], rest[MAXW:]
                nsi = nop.ins.sync_info
                if nsi is None:
                    nop.ins.sync_info = mybir.SyncInfo(on_wait=chunk, on_update=[])
                else:
                    nsi.on_wait = list(nsi.on_wait) + chunk
        self.nc.all_engine_barrier()
        assert self.sems is not None
        popped = self.nc._tile_sem_poison_stack.pop()
        assert popped is self._sem_poison
        self.nc.clear_and_free_semaphores(list(self.sems.allocated().values()))
        self.nc.all_engine_barrier()

    tile_mod.TileContext._drain_and_barrier = patched
    tile_mod.TileContext._drain_patched = True


def _patch_profile_hook():
    """Optional: register the axon NTFF profiling hook so trace=True works
    (used for timing; harmless no-op if unavailable)."""
    if "antenv.axon_hooks" in sys.modules:
        return
    try:
        sys.path.insert(0, "/root/.axon_site")
        from trn_agent_boot.trn_boot import _ntff_profile_via_ctypes

        hook = _ntff_profile_via_ctypes("/opt/axon/libaxon_pjrt.so")
        mod = types.ModuleType("antenv.axon_hooks")
        mod.get_axon_ntff_profile_hook = lambda: hook
        mod.set_axon_ntff_profile_hook = lambda h: None
        sys.modules["antenv.axon_hooks"] = mod
        import concourse.bass_utils as bu

        bu.upload_artifacts = lambda tmpdir: "local://" + tmpdir
    except Exception:
        pass


# --------------------------------------------------------------------------
# program construction (shared by all 8 cores; SPMD over input data)
# --------------------------------------------------------------------------

def _build_nc():
    import contextlib

    import concourse.bass as bass
    import concourse.mybir as mybir
    from concourse.tile import TileContext

    f32 = mybir.dt.float32
    f32r = mybir.dt.float32r
    bf16 = mybir.dt.bfloat16
    ALU = mybir.AluOpType
    AF = mybir.ActivationFunctionType

    nc = bass.Bass()

    # ---- DRAM I/O ----
    xT = nc.declare_dram_parameter("xT", [C, T], bf16, isOutput=False)
    xTo = nc.declare_dram_parameter("xTo", [C, OT], f32, isOutput=False)
    xTo16 = nc.declare_dram_parameter("xTo16", [C, OT], bf16, isOutput=False)
    wq = nc.declare_dram_parameter("wq", [C, C], bf16, isOutput=False)
    wk = nc.declare_dram_parameter("wk", [C, C], bf16, isOutput=False)
    wv = nc.declare_dram_parameter("wv", [C, C], bf16, isOutput=False)
    w1 = nc.declare_dram_parameter("w1", [C, F], bf16, isOutput=False)
    w2 = nc.declare_dram_parameter("w2", [F, C], bf16, isOutput=False)
    b1r = nc.declare_dram_parameter("b1r", [P, NB_F], f32, isOutput=False)
    b2r = nc.declare_dram_parameter("b2r", [P, NB_C], f32, isOutput=False)
    g1r = nc.declare_dram_parameter("g1r", [P, NB_C], f32, isOutput=False)
    be1r = nc.declare_dram_parameter("be1r", [P, NB_C], f32, isOutput=False)
    g2r = nc.declare_dram_parameter("g2r", [P, NB_C], f32, isOutput=False)
    be2r = nc.declare_dram_parameter("be2r", [P, NB_C], f32, isOutput=False)
    cmask = nc.declare_dram_parameter("cmask", [P, 64], bf16, isOutput=False)
    sel4h = nc.declare_dram_parameter("sel4h", [P, 2 * P], f32, isOutput=False)
    outT = nc.declare_dram_parameter("outT", [C, OT], f32, isOutput=True)

    xT_t = xT[:].rearrange("(n p) t -> n p t", p=P)
    xTo_t = xTo[:].rearrange("(n p) t -> n p t", p=P)
    xTo16_t = xTo16[:].rearrange("(n p) t -> n p t", p=P)
    wq_t = wq[:].rearrange("(n p) c -> n p c", p=P)
    wk_t = wk[:].rearrange("(n p) c -> n p c", p=P)
    wv_t = wv[:].rearrange("(n p) c -> n p c", p=P)
    w1_t = w1[:].rearrange("(n p) f -> n p f", p=P)
    w2_t = w2[:].rearrange("(n p) c -> n p c", p=P)
    outT_t = outT[:].rearrange("(n p) t -> n p t", p=P)

    def r(ap):
        """bitcast fp32 matmul operands to fp32r (full-rate at N>=256)"""
        return ap.bitcast(f32r)

    with TileContext(nc) as tc, contextlib.ExitStack() as ctx:
        const = ctx.enter_context(tc.tile_pool(name="const", bufs=1))
        p_xto = ctx.enter_context(tc.tile_pool(name="xto", bufs=1))
        p_a = ctx.enter_context(tc.tile_pool(name="attn_a", bufs=1))
        p_mlpw = ctx.enter_context(tc.tile_pool(name="mlpw", bufs=1))
        import contextlib as _ctl
        xt_stack = _ctl.ExitStack()
        p_xt = xt_stack.enter_context(tc.tile_pool(name="xt", bufs=1))

        # ---- constants ----
        ones_k = const.tile([P, 1], bf16, tag="ones_k", name="ones_k")
        nc.vector.memset(ones_k, 1.0)
        ones_kf = const.tile([P, 1], f32, tag="ones_kf", name="ones_kf")
        nc.vector.memset(ones_kf, 1.0)
        ones_kr = const.tile([P, 1], f32r, tag="ones_kr", name="ones_kr")
        with nc.allow_low_precision(reason="f32r ones column for LN stats"):
            nc.vector.tensor_copy(ones_kr, ones_kf)
        ones_bf = const.tile([1, P], f32, tag="ones_bf", name="ones_bf")
        nc.vector.memset(ones_bf, 1.0)
        ones_b = const.tile([1, P], f32r, tag="ones_b", name="ones_b")
        with nc.allow_low_precision(reason="f32r ones for 1cyc/row bcast"):
            nc.vector.tensor_copy(ones_b, ones_bf)
        # pair selectors (host-built): sel4[:, g, :] as lhsT routes the
        # denominator-recip row at partition 32*(2g+par) to output rows
        # par*64..par*64+63 (par = 0,1) in one broadcast matmul per pair.
        # Attention-only tiles -> p_xt pool (freed before the MLP phase).
        sel4f = p_xt.tile([P, 2, P], f32, tag="sel4f", name="sel4f")
        nc.scalar.dma_start(out=sel4f, in_=sel4h[:].rearrange("p (g m) -> p g m", g=2))
        sel4 = p_xt.tile([P, 2, P], f32r, tag="sel4", name="sel4")
        with nc.allow_low_precision(reason="f32r selector for den bcast"):
            nc.vector.tensor_copy(sel4, sel4f)
        # denominator staging: head-slot jj's row lives at partition 32*jj
        # (engine partition starts must be 32-aligned); one reciprocal
        # covers all 128 lanes at the cost of one row
        den4 = p_xt.tile([P, 512], f32, tag="den4", name="den4")
        nc.vector.memset(den4, 1.0)
        rec4 = p_xt.tile([P, 512], f32r, tag="rec4", name="rec4")
        eps_t = const.tile([1, 1], f32, tag="eps", name="eps_t")
        nc.vector.memset(eps_t, EPS)
        # dummy exp: pulls the ~2.7us exp table load into the startup DMA
        # wait instead of stalling the first attention softmax
        warm = const.tile([1, 1], f32, tag="warm", name="warm")
        nc.scalar.activation(out=warm, in_=eps_t, func=AF.Exp, scale=1.0)
        msk = const.tile([P, 64], bf16, tag="msk", name="msk")
        nc.scalar.dma_start(out=msk, in_=cmask[:])
        msk2 = bass.AP(
            tensor=msk.tensor, offset=msk.offset,
            ap=[list(msk.ap[0]), [0, 2], list(msk.ap[1])],
        )
        sb_b1 = const.tile([P, NB_F], f32, tag="b1", name="sb_b1")
        nc.scalar.dma_start(out=sb_b1, in_=b1r[:])
        sb_b2 = const.tile([P, NB_C], f32, tag="b2", name="sb_b2")
        nc.scalar.dma_start(out=sb_b2, in_=b2r[:])
        sb_g1 = const.tile([P, NB_C], f32, tag="g1", name="sb_g1")
        nc.scalar.dma_start(out=sb_g1, in_=g1r[:])
        sb_be1 = const.tile([P, NB_C], f32, tag="be1", name="sb_be1")
        nc.scalar.dma_start(out=sb_be1, in_=be1r[:])
        sb_g2 = const.tile([P, NB_C], f32, tag="g2", name="sb_g2")
        nc.scalar.dma_start(out=sb_g2, in_=g2r[:])
        sb_be2 = const.tile([P, NB_C], f32, tag="be2", name="sb_be2")
        nc.scalar.dma_start(out=sb_be2, in_=be2r[:])

        # ---- persistent activations ----
        # combined activation tiles: one DMA dispatch each (per-chunk
        # dispatches cost ~0.7us of queue time apiece at startup)
        xt_all = p_xt.tile([P, NB_C, T], bf16, tag="xt", name="xt")
        xto_all = p_xto.tile([P, NB_C, OT], f32, tag="xto", name="xto")
        xto16_all = p_xt.tile([P, NB_C, OT], bf16, tag="xto16", name="xto16")
        sb_xt = [xt_all[:, k, :] for k in range(NB_C)]
        sb_xto = [xto_all[:, k, :] for k in range(NB_C)]
        sb_xto16 = [xto16_all[:, k, :] for k in range(NB_C)]

        # attention output a^T, bf16 [128, OT] per pair-chunk
        sb_a = [
            p_a.tile([P, OT], bf16, tag=f"a{pc}", name=f"a{pc}")
            for pc in range(NPAIR)
        ]

        # MLP weight tiles: allocate now (address space), DMA later --
        # issuing these loads early would queue 9.4MB ahead of the weights
        # the first projections need and stall the PE for ~60us.
        sb_w1 = [
            p_mlpw.tile([P, F], bf16, tag=f"w1_{k}", name=f"w1_{k}")
            for k in range(NB_C)
        ]
        # w2 lives in a phase-B pool (opened after the attention tiles are
        # freed): its 36KB/partition would otherwise cap attention buffers

        # rotating projection-weight pool: one combined DMA per (w, pc),
        # prefetched one group ahead so group starts never wait on HBM
        # (lives on xt_stack: freed with the other attention-only tiles)
        p_wg = xt_stack.enter_context(tc.tile_pool(name="wqkv", bufs=2))
        wq_r = wq[:].rearrange("(k p) c -> p k c", p=P)
        wk_r = wk[:].rearrange("(k p) c -> p k c", p=P)
        wv_r = wv[:].rearrange("(k p) c -> p k c", p=P)

        def load_gw(hg):
            pcs_ = [hg * (HG // 2) + i for i in range(HG // 2)]
            wt = {}
            for i, pc in enumerate(pcs_):
                t = p_wg.tile([P, NB_C, P], bf16, tag=f"wk{i}",
                              name=f"wk{hg}_{i}")
                nc.sync.dma_start(out=t, in_=wk_r[:, :, pc * P : (pc + 1) * P])
                wt["k", pc] = t
            for i, pc in enumerate(pcs_):
                t = p_wg.tile([P, NB_C, P], bf16, tag=f"wq{i}",
                              name=f"wq{hg}_{i}")
                nc.sync.dma_start(out=t, in_=wq_r[:, :, pc * P : (pc + 1) * P])
                wt["q", pc] = t
            t = p_wg.tile([P, NB_C, HG * D], bf16, tag="wv", name=f"wv{hg}")
            d0 = 2 * pcs_[0] * D
            nc.sync.dma_start(out=t, in_=wv_r[:, :, d0 : d0 + HG * D])
            wt["v"] = t
            return wt

        # startup: group-0 weights race down the sync queue while the bulk
        # activation loads stream on the (otherwise idle) gpsimd DMA queue;
        # the k projection only needs the small xTo16 load, so it runs first
        gw = {0: load_gw(0)}
        xto16_r = xTo16[:].rearrange("(k p) t -> p k t", p=P)
        # two half-loads: the first k-projection group reads only columns
        # 0-511, so it can start while the second half's SWDGE descriptor
        # generation (~1.3us) overlaps the first half's transfer
        nc.gpsimd.dma_start(
            out=xto16_all[:, :, 0:512], in_=xto16_r[:, :, 0:512]
        )
        nc.gpsimd.dma_start(
            out=xto16_all[:, :, 512:1024], in_=xto16_r[:, :, 512:1024]
        )
        nc.gpsimd.dma_start(
            out=xt_all, in_=xT[:].rearrange("(k p) t -> p k t", p=P)
        )
        nc.gpsimd.dma_start(
            out=xto_all, in_=xTo[:].rearrange("(k p) t -> p k t", p=P)
        )

        # deferred normalize-finish closures (see norm_a/norm_b below)
        norm_q = []

        def flush_norm(psum_pool):
            while norm_q:
                norm_q.pop(0)(psum_pool)

        # ============================================================
        # Phase A: attention, in head groups of HG
        # ============================================================
        for hg in range(N_HG):
            pcs = [hg * (HG // 2) + i for i in range(HG // 2)]
            heads = [2 * pc + j for pc in pcs for j in range(2)]

            with contextlib.ExitStack() as gctx:
                p_qk = gctx.enter_context(tc.tile_pool(name=f"qk{hg}", bufs=1))
                p_v = gctx.enter_context(tc.tile_pool(name=f"v{hg}", bufs=1))
                p_ps = gctx.enter_context(
                    tc.tile_pool(name=f"ps{hg}", bufs=1, space="PSUM")
                )
                p_e = gctx.enter_context(tc.tile_pool(name=f"e{hg}", bufs=1))

                wt = gw.pop(hg)
                _sc_p = nc.enter_named_scope(f"proj{hg}", False)
                # ---- projections for this group ----
                q_t, k_t = {}, {}
                for pc in pcs:
                    q_t[pc] = p_qk.tile([P, T], bf16, tag=f"q{pc}", name=f"q{pc}")
                    k_t[pc] = p_qk.tile([P, OT], bf16, tag=f"k{pc}", name=f"k{pc}")
                # v for 4 heads: [128, 4, 65] per s-chunk (col 64 of each
                # head-slot = 1.0 for the softmax-denominator value row)
                v4 = []
                for sc in range(T // P):
                    vt = p_v.tile(
                        [P, HG, 65], bf16, tag=f"v4_{sc}", name=f"v4_{sc}"
                    )
                    nc.vector.memset(vt[:, :, 64:65], 1.0)
                    v4.append(vt)

                # k projection first: it only needs the small xTo16 load
                for pc in pcs:
                    wkl = wt["k", pc]
                    for t2 in range(OT // 512):
                        ps = p_ps.tile(
                            [P, 512], f32, tag="ps", bufs=2, name="ps_prk"
                        )
                        for k in range(NB_C):
                            nc.tensor.matmul(
                                ps,
                                wkl[:, k, :],
                                sb_xto16[k][:, t2 * 512 : (t2 + 1) * 512],
                                start=(k == 0),
                                stop=(k == NB_C - 1),
                            )
                        nc.vector.tensor_copy(
                            k_t[pc][:, t2 * 512 : (t2 + 1) * 512], ps
                        )
                for pc in pcs:
                    # q projection: full T
                    wql = wt["q", pc]
                    for t4 in range(T // 512):
                        ps = p_ps.tile(
                            [P, 512], f32, tag="ps", bufs=2, name="ps_prj"
                        )
                        for k in range(NB_C):
                            nc.tensor.matmul(
                                ps,
                                wql[:, k, :],
                                sb_xt[k][:, t4 * 512 : (t4 + 1) * 512],
                                start=(k == 0),
                                stop=(k == NB_C - 1),
                            )
                        nc.vector.tensor_copy(
                            q_t[pc][:, t4 * 512 : (t4 + 1) * 512], ps
                        )

                # v projection: full T, into per-head [128, 65] tiles
                wvl = wt["v"]
                for sc in range(T // P):
                    ps = p_ps.tile(
                        [P, HG * D], f32, tag="ps", bufs=2, name="ps_v"
                    )
                    for k in range(NB_C):
                        nc.tensor.matmul(
                            ps,
                            sb_xt[k][:, sc * P : (sc + 1) * P],
                            wvl[:, k, :],
                            start=(k == 0),
                            stop=(k == NB_C - 1),
                        )
                    nc.vector.tensor_copy(
                        v4[sc][:, :, 0:64],
                        ps[:].rearrange("p (h d) -> p h d", h=HG),
                    )

                nc.leave_named_scope(f"proj{hg}", _sc_p[0], False)
                if hg == 1:
                    for k in range(NB_C):
                        nc.sync.dma_start(out=sb_w1[k], in_=w1_t[k])
                if hg + 1 < N_HG:
                    gw[hg + 1] = load_gw(hg + 1)
                _sc_a = nc.enter_named_scope(f"attn{hg}", False)
                # ---- attention ----
                for tb in range(2):
                    nsc = 8 * tb + 8          # s-chunks for this own-block
                    av = {}
                    for h in heads:
                        av[h] = p_ps.tile(
                            [P, 512], f32, tag=f"av{h % HG}", name=f"av{h}"
                        )

                    def issue_av(pc, sc, c0, et, _nsc=nsc, _av=av):
                        for par in range(2):
                            h = 2 * pc + par
                            jj = heads.index(h)
                            nc.tensor.matmul(
                                _av[h][0:65, c0:512],
                                v4[sc][:, jj, :],
                                et[:, par, c0:512],
                                start=(sc == 0),
                                stop=(sc == _nsc - 1),
                            )

                    # software pipeline: AV matmuls for s-chunk sc issue
                    # after the NEXT chunk's score matmuls, so the in-order
                    # PE queue never stalls on the exp (ACT) latency.
                    pend = []
                    for sc in range(nsc):
                        c0 = max(0, 64 * sc - 512 * tb)   # first live t-col
                        for pc in pcs:
                            ps = p_ps.tile(
                                [P, 2, 512], f32, tag="ps", bufs=2,
                                name="ps_sc",
                            )
                            for par in range(2):
                                # K=64 row-tiled pair: both heads' score
                                # matmuls run concurrently on the PE array
                                nc.tensor.matmul(
                                    ps[:, par, c0:512],
                                    q_t[pc][par * 64 : par * 64 + 64,
                                            sc * P : (sc + 1) * P],
                                    k_t[pc][par * 64 : par * 64 + 64,
                                            tb * 512 + c0 : (tb + 1) * 512],
                                    start=True,
                                    stop=True,
                                    tile_position=(par * 64, 0),
                                )
                            et = p_e.tile(
                                [P, 2, 512], bf16, tag="exp", bufs=4, name="et"
                            )
                            nc.scalar.activation(
                                out=et[:, :, c0:512],
                                in_=ps[:, :, c0:512],
                                func=AF.Exp,
                                scale=SCALE,
                            )
                            if sc >= 8 * tb:   # causal boundary stripe
                                # multiplicative {0,1} mask post-exp: keeps
                                # the DVE op bf16 + SBUF-only (fast path)
                                nc.vector.tensor_tensor(
                                    et[:, :, c0 : c0 + 64],
                                    et[:, :, c0 : c0 + 64],
                                    msk2[:, :, 0:64],
                                    ALU.mult,
                                )
                            pend.append((pc, sc, c0, et))
                        while pend and pend[0][1] < sc:
                            issue_av(*pend.pop(0))
                        if sc == 6:
                            # finish the PREVIOUS block's normalize here:
                            # its reciprocal is long done, so the broadcast
                            # matmuls slot into the PE stream with no stall
                            flush_norm(p_ps)
                    for item in pend:
                        issue_av(*item)
                    # normalize part A (at block end): evacuate raw AV to
                    # sb_a (bf16) and the denominator rows into den4, then
                    # one batched reciprocal. This frees the AV psum banks
                    # within ~2us so the next block's accumulation starts
                    # immediately; the finish (norm_b) is deferred.
                    for jj, h in enumerate(heads):
                        pc, par = h // 2, h % 2
                        a_dst = sb_a[pc][par * 64 : par * 64 + 64,
                                         tb * 512 : (tb + 1) * 512]
                        d_dst = den4[32 * jj : 32 * jj + 1, :]
                        # split the 8 evacuation copies across ACT and DVE
                        # so each AV psum bank frees within ~0.7us of its
                        # last matmul (the next block's accumulation reuses
                        # the same 4 banks)
                        if jj < 2:
                            nc.scalar.copy(a_dst, av[h][0:64, 0:512])
                            nc.vector.tensor_copy(d_dst, av[h][64:65, 0:512])
                        else:
                            nc.vector.tensor_copy(a_dst, av[h][0:64, 0:512])
                            nc.scalar.copy(d_dst, av[h][64:65, 0:512])
                    with nc.allow_low_precision(reason="f32r recip bcast"):
                        nc.vector.reciprocal(rec4, den4)

                    def norm_b(psum_pool, _pcs=pcs, _tb=tb):
                        for g, pc in enumerate(_pcs):
                            den_ps = psum_pool.tile(
                                [P, 512], f32, tag="ps", bufs=2, name="den_ps"
                            )
                            nc.tensor.matmul(
                                den_ps, sel4[:, g, :], rec4,
                                start=True, stop=True,
                            )
                            den_sb = p_xt.tile(
                                [P, 512], bf16, tag="den_sb", bufs=4,
                                name="den_sb",
                            )
                            nc.scalar.copy(den_sb, den_ps)
                            sl_c = slice(_tb * 512, (_tb + 1) * 512)
                            for par in range(2):
                                sl_r = slice(par * 64, par * 64 + 64)
                                nc.vector.tensor_tensor(
                                    sb_a[pc][sl_r, sl_c],
                                    sb_a[pc][sl_r, sl_c],
                                    den_sb[sl_r, :],
                                    ALU.mult,
                                )
                            # fold the LN1 residual in here too: sb_a
                            # becomes x + a while attention still runs,
                            # so phase B starts straight at the LN1 stats
                            nc.vector.tensor_tensor(
                                sb_a[pc][:, sl_c],
                                sb_a[pc][:, sl_c],
                                sb_xto[pc][:, sl_c],
                                ALU.add,
                            )

                    norm_q.append(norm_b)

                # group ends: finish this group's tb=1 normalize inside the
                # group (needs an open PSUM pool); only the last group pays
                # a short recip wait here
                if hg + 1 == N_HG:
                    flush_norm(p_ps)
                nc.leave_named_scope(f"attn{hg}", _sc_a[0], False)

        xt_stack.close()   # free xT / xTo16 before the MLP pools open

        # ============================================================
        # Phase B: residual + LN1 + MLP + residual + LN2, per tb
        # ============================================================
        with contextlib.ExitStack() as mctx:
            mctx.enter_context(nc.named_scope("mlp"))
            p_w2 = mctx.enter_context(tc.tile_pool(name="w2p", bufs=1))
            sb_w2 = [
                p_w2.tile([P, C], bf16, tag=f"w2_{m}", name=f"w2_{m}")
                for m in range(NB_F)
            ]
            for m in range(NB_F):
                nc.sync.dma_start(out=sb_w2[m], in_=w2_t[m])
            p_r1 = mctx.enter_context(tc.tile_pool(name="r1", bufs=1))
            p_ln = mctx.enter_context(tc.tile_pool(name="ln", bufs=1))
            p_tmp = mctx.enter_context(tc.tile_pool(name="tmp", bufs=1))
            p_st = mctx.enter_context(tc.tile_pool(name="st", bufs=1))
            p_psm = mctx.enter_context(
                tc.tile_pool(name="psm", bufs=1, space="PSUM")
            )
            p_h = mctx.enter_context(tc.tile_pool(name="hsb", bufs=1))

            # r1 lives as f32r so the LN mean-stats matmul can consume it
            # at full PE rate without an extra rounding copy (f32r keeps
            # 19 mantissa bits - ~1e-4 relative, far inside tolerance)
            r1 = [
                p_r1.tile([P, OT], f32r, tag=f"r1_{c}", name=f"r1_{c}")
                for c in range(NB_C)
            ]
            ln1 = [
                p_ln.tile([P, OT], bf16, tag=f"ln1_{c}", name=f"ln1_{c}")
                for c in range(NB_C)
            ]

            def layer_norm_stats(src_tiles, ones_st=ones_kr):
                """transposed LN part 1: stats via ones-matmuls, then the
                DVE/ACT scalar chain ending in the (slow, 3.3us) reciprocal.
                Returns (mu, rsg) row tiles."""
                mu_ps = p_psm.tile([1, 512], f32, tag="lnst", bufs=2, name="mu_ps")
                sq_ps = p_psm.tile([1, 512], f32, tag="lnst", bufs=2, name="sq_ps")
                for c in range(NB_C):
                    # squares on ACT (Square lives in every table set);
                    # mean stats straight off the source matmul
                    s = p_tmp.tile([P, 512], bf16, tag="sqt", bufs=2, name="sqt")
                    nc.scalar.activation(
                        out=s, in_=src_tiles[c], func=AF.Square, scale=1.0
                    )
                    with nc.allow_low_precision(reason="f32r LN mean stats"):
                        nc.tensor.matmul(
                            mu_ps, ones_st, src_tiles[c],
                            start=(c == 0), stop=(c == NB_C - 1),
                        )
                    nc.tensor.matmul(
                        sq_ps, ones_k, s,
                        start=(c == 0), stop=(c == NB_C - 1),
                    )
                mu = p_st.tile([1, 512], f32r, tag="mu_s", bufs=2, name="mu")
                with nc.allow_low_precision(reason="f32r stats for 1cyc/row bcast"):
                    nc.vector.tensor_scalar_mul(mu, mu_ps, 1.0 / C)
                sq = p_st.tile([1, 512], f32, tag="sq_s", bufs=2, name="sq")
                nc.vector.tensor_scalar_mul(sq, sq_ps, 1.0 / C)
                var = p_st.tile([1, 512], f32, tag="var", bufs=2, name="var")
                nc.vector.tensor_tensor(var, mu, mu, ALU.mult)
                nc.vector.tensor_tensor(var, sq, var, ALU.subtract)
                sd = p_st.tile([1, 512], f32, tag="sd", bufs=2, name="sd")
                nc.scalar.activation(
                    out=sd, in_=var, func=AF.Sqrt, bias=eps_t, scale=1.0
                )
                rsg = p_st.tile([1, 512], f32r, tag="rsg", bufs=2, name="rsg")
                with nc.allow_low_precision(reason="f32r rsig for 1cyc/row bcast"):
                    nc.vector.reciprocal(rsg, sd)
                return mu, rsg

            def layer_norm_finish(src_tiles, mu, rsg, gt, bt, out_cb,
                                  split=False):
                """transposed LN part 2: broadcast matmuls (issue these only
                once the reciprocal has had time to run, or the PE queue
                stalls behind them) + per-chunk normalize. split=True farms
                half the d1 chain to GPSIMD for the tail."""
                mu_b = p_psm.tile([P, 512], f32, tag="lnbc", bufs=2, name="mu_b")
                nc.tensor.matmul(mu_b, ones_b, mu, start=True, stop=True)
                rs_b = p_psm.tile([P, 512], f32, tag="lnbc", bufs=2, name="rs_b")
                nc.tensor.matmul(rs_b, ones_b, rsg, start=True, stop=True)
                mu_bs = p_tmp.tile([P, 512], f32, tag="mu_bs", bufs=1, name="mu_bs")
                nc.scalar.copy(mu_bs, mu_b)
                rs_bs = p_tmp.tile([P, 512], f32, tag="rs_bs", bufs=1, name="rs_bs")
                nc.scalar.copy(rs_bs, rs_b)
                for c in range(NB_C):
                    eng = nc.gpsimd if (split and c % 3 == 2) else nc.vector
                    d1 = p_tmp.tile([P, 512], f32, tag="d1", bufs=3, name="d1")
                    eng.tensor_tensor(d1, src_tiles[c], mu_bs, ALU.subtract)
                    eng.tensor_tensor(d1, d1, rs_bs, ALU.mult)
                    out_cb(c, d1, gt, bt)

            def layer_norm_T(src_tiles, gt, bt, out_cb, split=False):
                mu, rsg = layer_norm_stats(src_tiles)
                layer_norm_finish(src_tiles, mu, rsg, gt, bt, out_cb,
                                  split=split)

            # LN1 for BOTH halves first (the x + a residual was already
            # folded into sb_a during attention): stats for both halves are
            # issued before either finish so the broadcast matmuls never
            # sit in the PE queue waiting on the 3.3us reciprocal.
            ln1_parts = []
            for tb in range(2):
                sl = slice(tb * 512, (tb + 1) * 512)
                r1v = [sb_a[c][:, sl] for c in range(NB_C)]

                def ln1_out(c, d2, gt, bt, _sl=sl):
                    nc.vector.tensor_scalar(
                        out=ln1[c][:, _sl], in0=d2,
                        scalar1=gt[:, c : c + 1], scalar2=bt[:, c : c + 1],
                        op0=ALU.mult, op1=ALU.add,
                    )
                    # f32r copy for the post-LN residual (r1 is dead: reuse)
                    with nc.allow_low_precision(reason="f32r residual"):
                        nc.gpsimd.tensor_scalar(
                            out=r1[c][:, _sl], in0=d2,
                            scalar1=gt[:, c : c + 1], scalar2=bt[:, c : c + 1],
                            op0=ALU.mult, op1=ALU.add,
                        )

                ln1_parts.append(
                    (r1v, layer_norm_stats(r1v, ones_st=ones_k), ln1_out)
                )
            for r1v_, (mu_, rsg_), cb_ in ln1_parts:
                layer_norm_finish(r1v_, mu_, rsg_, sb_g1, sb_be1, cb_)

            # LN2(tb) is deferred until after pass1(tb+1)'s matmuls are in
            # the PE queue: its stats matmuls wait on the GPSIMD y-adds, and
            # issuing them eagerly would stall everything behind them. The
            # finish (broadcast matmuls) waits further still, so the PE
            # never queues behind the reciprocal.
            ln2_pend = None
            ln2_stats = None
            for tb in range(2):
                sl = slice(tb * 512, (tb + 1) * 512)

                # MLP pass 1: h[m] = gelu(W1.T ln1 + b1) -> SBUF
                h_sb = []
                for m in range(NB_F):
                    h_ps = p_psm.tile([P, 512], f32, tag="h_ps", bufs=2, name="h_ps")
                    for k in range(NB_C):
                        nc.tensor.matmul(
                            h_ps,
                            sb_w1[k][:, m * P : (m + 1) * P],
                            ln1[k][:, sl],
                            start=(k == 0),
                            stop=(k == NB_C - 1),
                        )
                    hs = p_h.tile([P, 512], bf16, tag=f"h{m}", name=f"h{m}")
                    nc.scalar.activation(
                        out=hs, in_=h_ps, func=AF.Gelu,
                        bias=sb_b1[:, m : m + 1], scale=1.0,
                    )
                    h_sb.append(hs)

                # LN2(prev) stats AFTER the last gelu: its Sqrt would
                # otherwise force two ~2.7us ACT table-set reloads in the
                # middle of this pass's gelu stream (Square needs no
                # switch, so the stats matmuls were free to wait)
                if ln2_pend is not None:
                    ln2_stats = layer_norm_stats(ln2_pend[0])

                # MLP pass 2: y[c] = sum_m W2[m,c].T h[m];  r2 = y + b2 + r1
                r2v = []
                for c in range(NB_C):
                    y_ps = p_psm.tile([P, 512], f32, tag="y_ps", bufs=2, name="y_ps")
                    for m in range(NB_F):
                        nc.tensor.matmul(
                            y_ps,
                            sb_w2[m][:, c * P : (c + 1) * P],
                            h_sb[m],
                            start=(m == 0),
                            stop=(m == NB_F - 1),
                        )
                    y_sb = p_h.tile([P, 512], f32, tag="y_sb", bufs=1, name="y_sb")
                    nc.scalar.activation(
                        out=y_sb, in_=y_ps, func=AF.Identity,
                        bias=sb_b2[:, c : c + 1], scale=1.0,
                    )
                    # alternate engines: this add chain gates the LN2 stats
                    eng = nc.vector if c % 2 == 0 else nc.gpsimd
                    with nc.allow_low_precision(reason="f32r residual"):
                        eng.tensor_tensor(
                            r1[c][:, sl], y_sb, r1[c][:, sl], ALU.add
                        )
                    r2v.append(r1[c][:, sl])
                    if c == 3 and ln2_pend is not None:
                        # finish once the reciprocal has had ~3 matmul
                        # groups of time to complete
                        layer_norm_finish(
                            ln2_pend[0], *ln2_stats, sb_g2, sb_be2,
                            ln2_pend[1],
                        )
                        ln2_pend = None

                def ln2_out(c, d2, gt, bt, _sl=sl, _last=(tb == 1)):
                    # stage the output in the long-dead xto residual tile
                    # (same shape/dtype; each (c, half) region is distinct,
                    # so there is no buffer rotation to wait on)
                    o = sb_xto[c][:, _sl]
                    eng = nc.gpsimd if (_last and c % 2 == 1) else nc.vector
                    eng.tensor_scalar(
                        out=o, in0=d2,
                        scalar1=gt[:, c : c + 1], scalar2=bt[:, c : c + 1],
                        op0=ALU.mult, op1=ALU.add,
                    )
                    dma = nc.gpsimd if (_last and c % 2 == 1) else nc.sync
                    dma.dma_start(out=outT_t[c][:, _sl], in_=o)

                ln2_pend = (r2v, ln2_out)
            # final half: no more matmuls to hide behind -- split the d1 /
            # output chain across DVE and GPSIMD to shorten the tail
            layer_norm_T(ln2_pend[0], sb_g2, sb_be2, ln2_pend[1], split=True)

    return nc


def _spill_excess_waits(nc, maxw=2):
    """walrus (this build) caps sync-wait commands per instruction. Move
    excess waits onto freshly inserted same-engine nops placed immediately
    before the over-limit instruction (same engine stream => the waits
    still complete before it executes)."""
    import copy

    import concourse.bass as bass
    import concourse.mybir as mybir

    scratch = bass.Bass()
    tpl = scratch.sync.nop(nofuse=True).ins
    ctr = [0]

    def mknop(engine, waits):
        n = copy.deepcopy(tpl)
        ctr[0] += 1
        n.name = f"I-spill{ctr[0]}"
        n.engine = engine
        n.sync_info = mybir.SyncInfo(on_wait=list(waits), on_update=[])
        return n

    fn = nc.m.functions[0]
    for bb in fn.blocks:
        changed = False
        out = []
        for inst in bb.instructions:
            si = inst.sync_info
            waits = list(si.on_wait) if si and si.on_wait else []
            nupd = len(si.on_update) if si and si.on_update else 0
            lim = max(0, maxw - nupd)   # waits + updates <= maxw total
            if len(waits) > lim:
                keep = waits[-lim:] if lim else []
                rest = waits[: len(waits) - lim]
                while rest:
                    chunk, rest = rest[:1], rest[1:]
                    out.append(mknop(inst.engine, chunk))
                si.on_wait = keep
                changed = True
            out.append(inst)
        if changed:
            bb.instructions = out


def _get_nc():
    if "nc" not in _compiled:
        _patch_tile_drain()
        _patch_profile_hook()
        nc = _build_nc()
        _spill_excess_waits(nc, maxw=2)
        _compiled["nc"] = nc
    return _compiled["nc"]


# --------------------------------------------------------------------------
# host-side sharding
# --------------------------------------------------------------------------

def _make_in_maps(x, Wq, Wk, Wv, ln1_g, ln1_b, W1, b1, W2, b2, ln2_g, ln2_b):
    x = np.asarray(x, np.float32)
    wq_s = np.ascontiguousarray(
        np.asarray(Wq, np.float32).transpose(1, 0, 2).reshape(C, C)
    ).astype(BF16)
    wk_s = np.ascontiguousarray(
        np.asarray(Wk, np.float32).transpose(1, 0, 2).reshape(C, C)
    ).astype(BF16)
    wv_s = np.ascontiguousarray(
        np.asarray(Wv, np.float32).transpose(1, 0, 2).reshape(C, C)
    ).astype(BF16)
    w1b = np.asarray(W1, np.float32).astype(BF16)
    w2b = np.asarray(W2, np.float32).astype(BF16)
    b1r = np.ascontiguousarray(np.asarray(b1, np.float32).reshape(NB_F, P).T)
    b2r = np.ascontiguousarray(np.asarray(b2, np.float32).reshape(NB_C, P).T)
    g1r = np.ascontiguousarray(np.asarray(ln1_g, np.float32).reshape(NB_C, P).T)
    be1r = np.ascontiguousarray(np.asarray(ln1_b, np.float32).reshape(NB_C, P).T)
    g2r = np.ascontiguousarray(np.asarray(ln2_g, np.float32).reshape(NB_C, P).T)
    be2r = np.ascontiguousarray(np.asarray(ln2_b, np.float32).reshape(NB_C, P).T)

    in_maps = []
    for core in range(N_CORES):
        b, g = core // 2, core % 2
        xb = x[b]                                # [T, C]
        xTa = np.ascontiguousarray(xb.T)         # [C, T]
        own = np.arange(g, T, 2)
        xo = np.ascontiguousarray(xb[own].T)     # [C, OT] f32
        ii = np.arange(P)[:, None]
        mm = np.arange(64)[None, :]
        cm = np.where(ii <= 2 * mm + g, 1.0, 0.0).astype(BF16)
        # sel[p, g, m] = 1 iff p == 32*(2g+par) and m in par's 64-row block
        sel = np.zeros((P, 2, P), np.float32)
        for g in range(2):
            for par in range(2):
                sel[32 * (2 * g + par), g, 64 * par : 64 * par + 64] = 1.0
        sel = sel.reshape(P, 2 * P)
        in_maps.append(
            {
                "xT": xTa.astype(BF16),
                "xTo": xo,
                "xTo16": xo.astype(BF16),
                "wq": wq_s,
                "wk": wk_s,
                "wv": wv_s,
                "w1": w1b,
                "w2": w2b,
                "b1r": b1r,
                "b2r": b2r,
                "g1r": g1r,
                "be1r": be1r,
                "g2r": g2r,
                "be2r": be2r,
                "cmask": cm,
                "sel4h": sel,
            }
        )
    return in_maps


def _assemble(results):
    out = np.empty((B, T, C), np.float32)
    for core in range(N_CORES):
        b, g = core // 2, core % 2
        own = np.arange(g, T, 2)
        out[b, own, :] = results[core]["outT"].T
    return out


def kernel(_trace=False, **inputs):
    from concourse.bass_utils import run_bass_kernel_spmd

    nc = _get_nc()
    in_maps = _make_in_maps(**inputs)
    res = run_bass_kernel_spmd(nc, in_maps, list(range(N_CORES)), trace=_trace)
    out = _assemble(res.results)
    if _trace:
        return out, res
    return out

